# revision 1
# baseline (speedup 1.0000x reference)
"""Trainium2 Bass kernel for nn_MultiHeadedAttention_257698038597.

Multi-headed attention with channels: query/key/value [B=2,T=512,C=8,D=512],
mask [B,T,T,1]; four Linear(512,512) layers. Sharding: data-parallel over the
16 (b,c) pairs -> 2 units per core across 8 cores.

Per-core program (SPMD; identical program, per-core data):
  - host pre-transposes activations to x^T [D,T] (contraction dim on
    partitions), pre-transposes the mask, and folds the v-bias into the output
    bias (softmax rows sum to 1): bo' = bv @ Wo + bo.
  - projections: qT/kT = Wq^T x + bq (fp32r matmuls, PSUM fp32 accum), v
    natural layout with a ones-column per head appended (66-stride blocks).
  - scores^T[s,t] per head with K=64 head pairs packed onto partition halves.
    The mask is applied additively by PRELOADING PSUM with (mask-1)*240 via a
    bf16 identity matmul, so exp(0.125*psum) = exp(scores/8) * mask-ish
    (masked entries decay by e^-30) -- no elementwise mask pass needed.
  - att@v with the ones-column producing the softmax normalizer as psum row
    64; normalize during PSUM eviction with reciprocal_approx_fast + multiply.
  - y = attT^T Wo with bo' (gpsimd partition-broadcast once) added on DVE
    during the PSUM eviction.
"""
import numpy as np

import concourse.bass as bass
import concourse.mybir as mybir
import concourse.tile as tile
from concourse import bacc
from concourse.bass import ts

P = 128
B, T, C, D = 2, 512, 8, 512
H, DK = 8, 64
KO = D // P             # 4 contraction chunks
U = 2                   # units (b,c pairs) per core
VS = 66                 # v_sb per-head stride: 64 v cols + 1 ones + 1 pad
NCORES = 8
MASK_SCALE = 240.0      # (mask-1)*240; exp scale 1/8 makes it -30 per masked

F32 = mybir.dt.float32
F32R = mybir.dt.float32r
BF16 = mybir.dt.bfloat16

MM_DT = F32R            # matmul operand dtype
P_DT = F32R             # attention-probability (exp output) dtype

EXP = mybir.ActivationFunctionType.Exp
MUL = mybir.AluOpType.mult


def build_nc(repeat=1):
    nc = bacc.Bacc("TRN2", target_bir_lowering=False, debug=False)

    xqt = nc.dram_tensor("xqt", [U, P, KO, T], MM_DT, kind="ExternalInput")
    xkt = nc.dram_tensor("xkt", [U, P, KO, T], MM_DT, kind="ExternalInput")
    xvt = nc.dram_tensor("xvt", [U, P, KO, T], MM_DT, kind="ExternalInput")
    # mask bias (mask-1)*240, transposed: [P, KO, T]
    mbias = nc.dram_tensor("mbias", [P, KO, T], BF16, kind="ExternalInput")
    wq = nc.dram_tensor("wq", [P, KO, D], MM_DT, kind="ExternalInput")
    wk = nc.dram_tensor("wk", [P, KO, D], MM_DT, kind="ExternalInput")
    wv = nc.dram_tensor("wv", [P, KO, D], MM_DT, kind="ExternalInput")
    wo = nc.dram_tensor("wo", [P, KO, D], MM_DT, kind="ExternalInput")
    bqd = nc.dram_tensor("bqd", [P, KO], F32, kind="ExternalInput")
    bkd = nc.dram_tensor("bkd", [P, KO], F32, kind="ExternalInput")
    bo2d = nc.dram_tensor("bo2d", [1, D], F32, kind="ExternalInput")
    iden = nc.dram_tensor("iden", [P, P], BF16, kind="ExternalInput")
    y = nc.dram_tensor("y", [U, KO, P, D], F32, kind="ExternalOutput")

    with tile.TileContext(nc) as tc:
        import contextlib
        with contextlib.ExitStack() as ctx:
            const = ctx.enter_context(tc.tile_pool(name="const", bufs=1))
            xt_pool = ctx.enter_context(tc.tile_pool(name="xt", bufs=2))
            qk_pool = ctx.enter_context(tc.tile_pool(name="qk", bufs=2))
            p_pool = ctx.enter_context(tc.tile_pool(name="pp", bufs=6))
            att_pool = ctx.enter_context(tc.tile_pool(name="att", bufs=2))
            nrm_pool = ctx.enter_context(tc.tile_pool(name="nrm", bufs=2))
            y_pool = ctx.enter_context(tc.tile_pool(name="y", bufs=3))
            ps_proj = ctx.enter_context(tc.tile_pool(name="psp", bufs=2, space="PSUM"))
            ps_sc = ctx.enter_context(tc.tile_pool(name="pssc", bufs=2, space="PSUM"))
            ps_av = ctx.enter_context(tc.tile_pool(name="psav", bufs=2, space="PSUM"))

            # constants; weight DMAs chunked per-ko so the first matmuls can
            # start as soon as their chunk lands
            wq_sb = const.tile([P, KO, D], MM_DT, tag="wq")
            wk_sb = const.tile([P, KO, D], MM_DT, tag="wk")
            wv_sb = const.tile([P, KO, D], MM_DT, tag="wv")
            wo_sb = const.tile([P, KO, D], MM_DT, tag="wo")
            mb_sb = const.tile([P, KO, T], BF16, tag="mb")
            id_sb = const.tile([P, P], BF16, tag="iden")
            bq_sb = const.tile([P, KO], F32, tag="bq")
            bk_sb = const.tile([P, KO], F32, tag="bk")
            bo2_sb = const.tile([1, D], F32, tag="bo2")
            bo2_bc = const.tile([P, D], F32, tag="bo2bc")
            for ko in range(KO):
                nc.sync.dma_start(out=wq_sb[:, ko, :], in_=wq[:, ko, :])
                nc.sync.dma_start(out=wk_sb[:, ko, :], in_=wk[:, ko, :])
                nc.sync.dma_start(out=wv_sb[:, ko, :], in_=wv[:, ko, :])
            nc.sync.dma_start(out=bq_sb, in_=bqd[:, :])
            nc.sync.dma_start(out=bk_sb, in_=bkd[:, :])
            nc.scalar.dma_start(out=id_sb, in_=iden[:, :])
            nc.sync.dma_start(out=wo_sb, in_=wo[:, :, :])
            nc.sync.dma_start(out=bo2_sb, in_=bo2d[:, :])
            nc.gpsimd.partition_broadcast(bo2_bc[:], bo2_sb[0:1, :])

            for _rep in range(repeat):
                att_units = []
                for u in range(U):
                    # ---- load transposed activations (scalar HWDGE ring, chunked)
                    xq_sb = xt_pool.tile([P, KO, T], MM_DT, tag="xq")
                    xk_sb = xt_pool.tile([P, KO, T], MM_DT, tag="xk")
                    xv_sb = xt_pool.tile([P, KO, T], MM_DT, tag="xv")
                    for ko in range(KO):
                        nc.scalar.dma_start(out=xq_sb[:, ko, :], in_=xqt[u, :, ko, :])
                    for ko in range(KO):
                        nc.scalar.dma_start(out=xk_sb[:, ko, :], in_=xkt[u, :, ko, :])
                    if u == 0:
                        nc.scalar.dma_start(out=mb_sb, in_=mbias[:, :, :])
                    for ko in range(KO):
                        nc.scalar.dma_start(out=xv_sb[:, ko, :], in_=xvt[u, :, ko, :])

                    # ---- projections
                    qT_sb = qk_pool.tile([P, KO, T], MM_DT, tag="qT")
                    kT_sb = qk_pool.tile([P, KO, T], MM_DT, tag="kT")
                    v_sb = qk_pool.tile([P, KO, H * VS], MM_DT, tag="v")
                    # ones columns (position 64 of each 66-wide head block; 65 = pad)
                    ones_view = bass.AP(
                        tensor=v_sb.tensor, offset=v_sb[:, 0, 64].offset,
                        ap=[list(v_sb[:].ap[0]), [H * VS, KO], [VS, H], [1, 2]])
                    nc.vector.memset(ones_view.bitcast(F32), 1.0)

                    for mo in range(KO):
                        psq = ps_proj.tile([P, T], F32, tag="psp")
                        for ko in range(KO):
                            nc.tensor.matmul(psq[:], wq_sb[:, ko, ts(mo, P)],
                                             xq_sb[:, ko, :],
                                             start=(ko == 0), stop=(ko == KO - 1))
                        nc.vector.tensor_scalar_add(qT_sb[:, mo, :], psq[:],
                                                    bq_sb[:, mo, None])
                    for mo in range(KO):
                        psk = ps_proj.tile([P, T], F32, tag="psp")
                        for ko in range(KO):
                            nc.tensor.matmul(psk[:], wk_sb[:, ko, ts(mo, P)],
                                             xk_sb[:, ko, :],
                                             start=(ko == 0), stop=(ko == KO - 1))
                        nc.vector.tensor_scalar_add(kT_sb[:, mo, :], psk[:],
                                                    bk_sb[:, mo, None])
                    for mo in range(KO):
                        psv = ps_proj.tile([P, T], F32, tag="psp")
                        for ko in range(KO):
                            nc.tensor.matmul(psv[:], xv_sb[:, ko, ts(mo, P)],
                                             wv_sb[:, ko, :],
                                             start=(ko == 0), stop=(ko == KO - 1))
                        # scatter into per-head 66-strided blocks: [P, H, DK] view
                        v_dst = bass.AP(
                            tensor=v_sb.tensor, offset=v_sb[:, mo, 0].offset,
                            ap=[list(v_sb[:].ap[0]), [VS, H], [1, DK]])
                        nc.vector.tensor_copy(out=v_dst,
                                              in_=psv[:].rearrange("p (h d) -> p h d", h=H))

                    # ---- attention, head pairs (heads 2g / 2g+1 live on partition
                    # halves 0-63 / 64-127 of kT/qT chunk g -> packed matmuls)
                    attT_k = [att_pool.tile([P, T], MM_DT, tag=f"attT{ko}",
                                            name=f"attT{ko}_u{u}")
                              for ko in range(KO)]
                    att_units.append(attT_k)
                    for g in range(4):
                        heads = (2 * g, 2 * g + 1)
                        p_tiles = []
                        for so in range(KO):
                            sc = ps_sc.tile([P, 2, T], F32, tag="sc")
                            # preload mask bias (bf16 identity matmuls), then
                            # the two K=64 scores matmuls back-to-back so their
                            # disjoint partition halves overlap in the PE array
                            # (DVE mask-add alternative measured worse: the
                            # fp32 TT pass makes DVE the bottleneck)
                            for j in range(2):
                                nc.tensor.matmul(sc[:, j, :], id_sb[:, :],
                                                 mb_sb[:, so, :],
                                                 start=True, stop=False)
                            for j, h in enumerate(heads):
                                lo = 64 * (h % 2)
                                nc.tensor.matmul(
                                    sc[:, j, :],
                                    kT_sb[lo:lo + 64, g, ts(so, P)],
                                    qT_sb[lo:lo + 64, g, :],
                                    start=False, stop=True)
                            pt = p_pool.tile([P, 2, T], P_DT, tag="p")
                            nc.scalar.activation(pt[:], sc[:], EXP, scale=0.125)
                            p_tiles.append(pt)

                        for j, h in enumerate(heads):
                            av = ps_av.tile([P, T], F32, tag="av")
                            for so in range(KO):
                                nc.tensor.matmul(
                                    av[0:65, :],
                                    v_sb[:, so, VS * h:VS * h + 65],
                                    p_tiles[so][:, j, :],
                                    start=(so == 0), stop=(so == KO - 1))
                            # normalizer: copy sums row to partition 0,
                            # reciprocal, gpsimd-replicate, multiply.
                            # (reciprocal_approx_fast reading PSUM directly
                            # produced garbage on hardware -- keep the copy.)
                            sums_sb = nrm_pool.tile([1, T], F32, tag="sums")
                            nc.vector.tensor_copy(out=sums_sb[0:1, :],
                                                  in_=av[64:65, :])
                            nc.vector.reciprocal_approx_fast(
                                out=sums_sb[0:1, :], in_=sums_sb[0:1, :])
                            bc = nrm_pool.tile([64, T], F32, tag="bc")
                            nc.gpsimd.partition_broadcast(bc[:], sums_sb[0:1, :])
                            lo = 64 * (h % 2)
                            nc.vector.tensor_tensor(
                                out=attT_k[g][lo:lo + 64, :],
                                in0=av[0:64, :], in1=bc[:], op=MUL)

                # ---- output projections, emitted after both units so they can
                # fill PE gaps during the other unit's attention
                for u in range(U):
                    attT_k = att_units[u]
                    for tc_i in range(KO):
                        psy = ps_proj.tile([P, T], F32, tag="psp")
                        for ko in range(KO):
                            nc.tensor.matmul(psy[:], attT_k[ko][:, ts(tc_i, P)],
                                             wo_sb[:, ko, :],
                                             start=(ko == 0), stop=(ko == KO - 1))
                        y_sb = y_pool.tile([P, D], F32, tag="y")
                        nc.vector.tensor_tensor(out=y_sb[:], in0=psy[:],
                                                in1=bo2_bc[:],
                                                op=mybir.AluOpType.add)
                        nc.sync.dma_start(out=y[u, tc_i, :, :], in_=y_sb[:])

    nc.compile()
    return nc


_NC_CACHE = {}


def _get_nc(repeat=1):
    if repeat not in _NC_CACHE:
        _NC_CACHE[repeat] = build_nc(repeat)
    return _NC_CACHE[repeat]


def _chunkT(x):
    """[T,D] fp32 -> x^T chunked [P, KO, T] (din = ko*128 + p)."""
    return np.ascontiguousarray(x.T.reshape(KO, P, T).transpose(1, 0, 2))


def _chunkW(w):
    """[D,D] (in,out) -> [P, KO, D]."""
    return np.ascontiguousarray(w.reshape(KO, P, D).transpose(1, 0, 2))


def make_in_maps(query, key, value, mask, Wq, bq, Wk, bk, Wv, bv, Wo, bo):
    import ml_dtypes
    bf16 = ml_dtypes.bfloat16
    query = np.asarray(query, np.float32)
    key = np.asarray(key, np.float32)
    value = np.asarray(value, np.float32)
    mask = np.asarray(mask)
    Wq, Wk, Wv, Wo = (np.asarray(w, np.float32) for w in (Wq, Wk, Wv, Wo))
    bq, bk, bv, bo = (np.asarray(b, np.float32) for b in (bq, bk, bv, bo))

    bo2 = (bv @ Wo + bo).astype(np.float32).reshape(1, D)
    wq_h, wk_h, wv_h, wo_h = _chunkW(Wq), _chunkW(Wk), _chunkW(Wv), _chunkW(Wo)
    bq_h = np.ascontiguousarray(bq.reshape(KO, P).T)
    bk_h = np.ascontiguousarray(bk.reshape(KO, P).T)
    iden = np.eye(P, dtype=bf16)

    in_maps = []
    for core in range(NCORES):
        b = core // 4
        cs = [2 * (core % 4), 2 * (core % 4) + 1]
        xq = np.stack([_chunkT(query[b, :, c, :]) for c in cs])
        xk = np.stack([_chunkT(key[b, :, c, :]) for c in cs])
        xv = np.stack([_chunkT(value[b, :, c, :]) for c in cs])
        # mask bias: (mask^T - 1) * 240, [P, KO, T]
        mb = _chunkT((mask[b, :, :, 0].astype(np.float32) - 1.0) * MASK_SCALE)
        mb2 = np.ascontiguousarray(mb).astype(bf16)
        in_maps.append({
            "xqt": xq, "xkt": xk, "xvt": xv, "mbias": mb2,
            "wq": wq_h, "wk": wk_h, "wv": wv_h, "wo": wo_h,
            "bqd": bq_h, "bkd": bk_h, "bo2d": bo2, "iden": iden,
        })
    return in_maps


def assemble(results):
    out = np.empty((B, T, C, D), np.float32)
    for core, res in enumerate(results):
        b = core // 4
        cs = [2 * (core % 4), 2 * (core % 4) + 1]
        yv = res["y"]  # [U, KO, P, D]
        for u, c in enumerate(cs):
            out[b, :, c, :] = yv[u].reshape(T, D)
    return out


def kernel(**inputs):
    from concourse.bass_utils import run_bass_kernel_spmd
    nc = _get_nc()
    in_maps = make_in_maps(**inputs)
    res = run_bass_kernel_spmd(nc, in_maps, core_ids=list(range(NCORES)))
    return assemble(res.results)


if __name__ == "__main__":
    # single-core sim check of core 0 (units b=0, c=0,1)
    import jax
    jax.config.update("jax_platforms", "cpu")
    import sys
    sys.path.insert(0, "/root/problem")
    import reference

    inp = {k: np.asarray(v) for k, v in reference.setup_inputs().items()}
    ref = np.asarray(reference.reference(**inp))

    from concourse.bass_interp import CoreSim
    nc = _get_nc()
    in_maps = make_in_maps(**inp)
    core = 0
    sim = CoreSim(nc)
    sim.assign_tensors(in_maps[core])
    sim.simulate()
    yv = sim.tensor("y")
    b = core // 4
    cs = [2 * (core % 4), 2 * (core % 4) + 1]
    for u, c in enumerate(cs):
        got = yv[u].reshape(T, D)
        want = ref[b, :, c, :]
        err = np.abs(got - want)
        print(f"core0 unit{u} (b={b},c={c}): absmax={err.max():.3e} "
              f"rel={err.max()/np.abs(want).max():.3e}")

